# revision 1
# baseline (speedup 1.0000x reference)
"""DenseGAT layer on 8 trn2 NeuronCores.

Math (per batch b, head t, query node i, source node j):
    z_ij = src_i + dst_j
    W_ij = adj_ij * exp(leakyrelu_0.2(z_ij));  out_i = (W @ h)_i / (W @ 1)_i

Key identity: exp(lrelu(z)) = max(e^z, e^{0.2z}) and each branch factorizes:
    e^z = e^{src_i} * e^{dst_j},  e^{0.2z} = e^{0.2 src_i} * e^{0.2 dst_j}
With M1 = 1[z>=0]*adj (the only N^2-scale elementwise tensor, built in one
fused DVE scalar_tensor_tensor pass per head) and M2 = adj - M1:
    num_i = e^{src_i} * [ (M1 @ b.h)_i + e^{-0.8 src_i} * ((adj - M1) @ d.h)_i ]
where b = e^{dst}, d = e^{0.2 dst}. The e^{src_i} row factor cancels in the
softmax ratio, so with r_i = e^{-0.8 src_i}:
    out = (T1 + r*(T2)) rows 1..64 / row 0,  T1 = M1 @ [b | b.h],
    T2 = adj @ [d | d.h] - M1 @ [d | d.h]   (second term via negated weights)
All j-contraction runs on the tensor engine with j on partitions; adj is fed
pre-transposed (adjT[j, i]) and as bf16 ({0,1} exact) from the host.

Sharding: core c -> batch c//4, query rows (c%4)*1024..+1024. Each core
computes h for all 4096 source nodes (cheap) and its own 1024 output rows.
"""

import numpy as np
import ml_dtypes
from contextlib import ExitStack

import concourse.bass as bass
import concourse.mybir as mybir
import concourse.tile as tile
from concourse.bass import ts, ds
from concourse.bass_utils import run_bass_kernel_spmd
from concourse.masks import make_identity
from concourse.vector_clock import ScopedClock

B, N, IN = 2, 4096, 256
H, D = 4, 64
IBLK = 1024          # query rows per core
CH = N // 128        # 32 j-chunks
OCH = IBLK // 128    # 8 own-row chunks
NHALF = IBLK // 512  # psum halves of the i range

F32 = mybir.dt.float32
BF16 = mybir.dt.bfloat16
FT = mybir.ActivationFunctionType
OP = mybir.AluOpType

LAST_RESULT = None  # BassKernelResults of the most recent run (for test harness)


def _install_drain_split(maxw=1):
    """This walrus build rejects instructions with more than ~2 sem waits
    ("Too many sync wait commands"). Tile's kernel-tail drain waits on every
    proc's final tick in a single instruction; split it into a chain of SP
    nops carrying one wait each."""
    if getattr(tile.TileContext, "_drain_split_installed", False):
        return

    def _split_drain_and_barrier(self, tick_clock, wait_clock):
        nc = self.nc
        probe = nc.sync.nop(nofuse=True)
        wait_clock.add_sem_waits(probe.ins, ScopedClock({None: tick_clock.global_clock}))
        si = probe.ins.sync_info
        waits = list(si.on_wait) if si is not None else []
        if len(waits) > maxw:
            probe.ins.sync_info = mybir.SyncInfo(
                on_wait=waits[:maxw], on_update=list(si.on_update)
            )
            for i in range(maxw, len(waits), maxw):
                extra = nc.sync.nop(nofuse=True)
                extra.ins.sync_info = mybir.SyncInfo(
                    on_wait=waits[i:i + maxw], on_update=[]
                )
        nc.sync.drain()
        nc.all_engine_barrier()
        assert self.sems is not None
        popped = nc._tile_sem_poison_stack.pop()
        assert popped is self._sem_poison
        nc.clear_and_free_semaphores(list(self.sems.allocated().values()))
        nc.all_engine_barrier()

    tile.TileContext._drain_and_barrier = _split_drain_and_barrier
    tile.TileContext._drain_split_installed = True


def _split_excess_waits(nc, maxw=1):
    """Move excess sem-waits (beyond maxw per instruction) onto same-engine
    NoOps inserted immediately before the instruction. The engine blocks on
    the nops first, so semantics are unchanged; this walrus build rejects
    instructions carrying more than a couple of waits."""
    cnt = 0
    tpb = {mybir.EngineType.PE, mybir.EngineType.Activation, mybir.EngineType.Pool,
           mybir.EngineType.DVE, mybir.EngineType.SP}
    for f in nc.m.functions:
        for bb in f.blocks:
            out = []
            changed = False
            for inst in bb.instructions:
                si = getattr(inst, "sync_info", None)
                waits = list(si.on_wait) if si is not None else []
                if len(waits) > maxw and inst.engine in tpb:
                    changed = True
                    nlead = len(waits) - maxw
                    for k in range(0, nlead, maxw):
                        nop = mybir.InstNoOp(
                            name=f"wsplit{cnt}", engine=inst.engine, ins=[], outs=[],
                            sync_info=mybir.SyncInfo(
                                on_wait=waits[k:min(k + maxw, nlead)], on_update=[]))
                        cnt += 1
                        nc.register_instruction(nop, overwrite=True)
                        out.append(nop)
                    inst.sync_info = mybir.SyncInfo(
                        on_wait=waits[nlead:], on_update=list(si.on_update))
                out.append(inst)
            if changed:
                bb.instructions = out
    return cnt


def build_bass():
    _install_drain_split()
    nc = bass.Bass("TRN2", target_bir_lowering=False, debug=False, num_devices=1)

    adjT = nc.dram_tensor("adjT", [CH, 128, IBLK], BF16, kind="ExternalInput")
    xT = nc.dram_tensor("xT", [2, 128, N], F32, kind="ExternalInput")
    xTo = nc.dram_tensor("xTo", [2, 128, IBLK], F32, kind="ExternalInput")
    wtp = nc.dram_tensor("wtp", [2, 128, IN + 8 + 128], F32, kind="ExternalInput")
    outT = nc.dram_tensor("outT", [H * D, IBLK], F32, kind="ExternalOutput")

    with ExitStack() as ctx:
        tc = ctx.enter_context(tile.TileContext(nc))
        const = ctx.enter_context(tc.tile_pool(name="const", bufs=1))

        ident = const.tile([128, 128], F32, tag="ident")
        make_identity(nc, ident[:])

        adjT_sb = const.tile([128, CH, IBLK], BF16, tag="adjT")
        for c in range(CH):
            nc.sync.dma_start(adjT_sb[:, c, :], adjT.ap()[c])

        h_sb = const.tile([128, CH, H, D], BF16, tag="h")
        Vb = const.tile([128, CH, H, D + 1], BF16, tag="Vb")
        Vd = const.tile([128, CH, H, D + 1], BF16, tag="Vd")
        nVd = const.tile([128, CH, H, D + 1], BF16, tag="nVd")
        bcolb = const.tile([128, CH, H, 1], BF16, tag="bcolb")
        dcolb = const.tile([128, CH, H, 1], BF16, tag="dcolb")
        ndcolb = const.tile([128, CH, H, 1], BF16, tag="ndcolb")
        ndst = const.tile([128, CH, H], F32, tag="ndst")
        srow = const.tile([128, IBLK], BF16, tag="srow")
        rrow = const.tile([128, IBLK], F32, tag="rrow")

        with (
            tc.tile_pool(name="xin", bufs=1) as xin,
            tc.tile_pool(name="pps", bufs=2, space="PSUM") as pps,
            tc.tile_pool(name="ppt", bufs=2, space="PSUM") as ppt,
        ):
            xT_sb = [xin.tile([128, N], F32, tag=f"xT{k}", name=f"xTsb{k}") for k in range(2)]
            xTo_sb = [xin.tile([128, IBLK], F32, tag=f"xTo{k}", name=f"xTosb{k}") for k in range(2)]
            wtp_sb = [xin.tile([128, IN + 8 + 128], F32, tag=f"wtp{k}", name=f"wtpsb{k}") for k in range(2)]
            sdo = xin.tile([128, OCH, 128], F32, tag="sdo")
            for k in range(2):
                nc.sync.dma_start(xT_sb[k][:], xT.ap()[k])
                nc.sync.dma_start(xTo_sb[k][:], xTo.ap()[k])
                nc.sync.dma_start(wtp_sb[k][:], wtp.ap()[k])

            # h = x @ W^T and dst logits, for all 4096 source nodes
            for c in range(CH):
                ph = pps.tile([128, IN + 8], F32, tag="ph")
                for k in range(2):
                    nc.tensor.matmul(ph[:], xT_sb[k][:, ts(c, 128)], wtp_sb[k][:, 0:IN + 8],
                                     start=(k == 0), stop=(k == 1))
                nc.scalar.copy(h_sb[:, c, :, :], ph[:, 0:IN])
                nc.scalar.activation(bcolb[:, c, :, 0], ph[:, IN + 4:IN + 8], FT.Exp)
                nc.scalar.activation(dcolb[:, c, :, 0], ph[:, IN + 4:IN + 8], FT.Exp, scale=0.2)
                nc.scalar.activation(ndst[:, c, :], ph[:, IN + 4:IN + 8], FT.Copy, scale=-1.0)

            # src logits for this core's own 1024 query rows. The host packs
            # P_own so head t's src column lands at index 32t; after the PE
            # transpose each head's src row sits at the 32-aligned partition
            # 32t (SBUF APs cannot start at arbitrary partitions).
            for oc in range(OCH):
                pso = pps.tile([128, 128], F32, tag="pso")
                for k in range(2):
                    nc.tensor.matmul(pso[:], xTo_sb[k][:, ts(oc, 128)],
                                     wtp_sb[k][:, IN + 8:IN + 8 + 128],
                                     start=(k == 0), stop=(k == 1))
                nc.scalar.copy(sdo[:, oc, :], pso[:])
                pst = ppt.tile([128, 128], F32, tag="pst")
                nc.tensor.transpose(pst[:], sdo[:, oc, :], ident[:])
                for t in range(H):
                    nc.scalar.copy(srow[32 * t:32 * t + 1, ts(oc, 128)], pst[32 * t:32 * t + 1, :])
                    nc.scalar.activation(rrow[32 * t:32 * t + 1, ts(oc, 128)],
                                         pst[32 * t:32 * t + 1, :], FT.Exp, scale=-0.8)

            # per-source-node weight matrices [e^dst | e^dst * h] etc, one
            # broadcast-AP tensor op per matrix
            nc.vector.tensor_scalar_mul(ndcolb[:], dcolb[:], -1.0)
            nc.vector.tensor_copy(Vb[:, :, :, D], bcolb[:, :, :, 0])
            nc.vector.tensor_copy(Vd[:, :, :, D], dcolb[:, :, :, 0])
            nc.vector.tensor_copy(nVd[:, :, :, D], ndcolb[:, :, :, 0])
            for vt, colt in ((Vb, bcolb), (Vd, dcolb), (nVd, ndcolb)):
                _, cb = bass.broadcast_tensor_aps(vt[:, :, :, 0:D], colt[:, :, :, :])
                nc.vector.tensor_tensor(vt[:, :, :, 0:D], h_sb[:, :, :, :], cb, OP.mult)

        def bcast(dst_ap, src_row_ap):
            # DMA-broadcast one SBUF row across partitions: the repeat is a
            # stride-0 *free* dim on the source (partition dims must have
            # nonzero step), iterated in the same order as the dest's
            # partition dim so the element streams line up.
            lay = [list(src_row_ap.ap[0]), [0, dst_ap.shape[0]]] + [
                list(dims) for dims in src_row_ap.ap[1:]]
            src_b = bass.AP(src_row_ap.tensor, src_row_ap.offset, lay)
            nc.sync.dma_start(dst_ap, src_b)

        with (
            tc.tile_pool(name="mm", bufs=2, space="PSUM") as mps,
            tc.tile_pool(name="m1p", bufs=5) as m1p,
            tc.tile_pool(name="stp", bufs=3) as stp,
            tc.tile_pool(name="epp", bufs=2) as epp,
            tc.tile_pool(name="outp", bufs=2) as outp,
            tc.tile_pool(name="sbbp", bufs=1) as sbbp,
        ):
            sbb = [sbbp.tile([128, IBLK], BF16, tag=f"sbb{t}", name=f"sbb{t}") for t in range(H)]
            for t in range(H):
                bcast(sbb[t][:], srow[32 * t:32 * t + 1, :])

            for t in range(H):
                T1 = mps.tile([D + 1, IBLK], F32, tag="T1")
                T2 = mps.tile([D + 1, IBLK], F32, tag="T2")
                for c in range(CH):
                    m1 = m1p.tile([128, IBLK], BF16, tag="m1")
                    st = stp.tile([128, IBLK], BF16, tag="st")
                    nc.vector.tensor_single_scalar(st[:], sbb[t][:],
                                                   ndst[:, c, t:t + 1], OP.is_ge)
                    nc.vector.tensor_mul(m1[:], st[:], adjT_sb[:, c, :])
                    for half in range(NHALF):
                        sl = ds(half * 512, 512)
                        nc.tensor.matmul(T2[:, sl], Vd[:, c, t, :], adjT_sb[:, c, sl],
                                         start=(c == 0), stop=False)
                        nc.tensor.matmul(T1[:, sl], Vb[:, c, t, :], m1[:, sl],
                                         start=(c == 0), stop=(c == CH - 1))
                        nc.tensor.matmul(T2[:, sl], nVd[:, c, t, :], m1[:, sl],
                                         start=False, stop=(c == CH - 1))
                for half in range(NHALF):
                    # epilogue off the DVE: ACT drains PSUM, gpsimd combines;
                    # DVE keeps only the (vector-engine-only) reciprocal
                    sl = ds(half * 512, 512)
                    rb = epp.tile([D + 1, 512], F32, tag="rb")
                    bcast(rb[:], rrow[32 * t:32 * t + 1, sl])
                    s1 = epp.tile([D + 1, 512], F32, tag="s1")
                    nc.scalar.copy(s1[:], T1[:, sl])
                    s2 = epp.tile([D + 1, 512], F32, tag="s2")
                    nc.scalar.copy(s2[:], T2[:, sl])
                    v = epp.tile([D + 1, 512], F32, tag="v")
                    nc.gpsimd.tensor_mul(v[:], rb[:], s2[:])
                    num = epp.tile([D + 1, 512], F32, tag="num")
                    nc.gpsimd.tensor_add(num[:], v[:], s1[:])
                    rec = epp.tile([D + 1, 512], F32, tag="rec")
                    nc.vector.reciprocal(rec[D:D + 1, :], num[D:D + 1, :])
                    rb2 = epp.tile([D, 512], F32, tag="rb2")
                    bcast(rb2[:], rec[D:D + 1, :])
                    o = outp.tile([D, 512], F32, tag="o")
                    nc.gpsimd.tensor_mul(o[:], num[0:D, :], rb2[:])
                    nc.sync.dma_start(outT.ap()[ts(t, D), sl], o[:])
    _split_excess_waits(nc)
    return nc


_CACHED = None


def _get_bass():
    global _CACHED
    if _CACHED is None:
        _CACHED = build_bass()
    return _CACHED


def _prep_inputs(x, adj, W_proj, attn_src, attn_dst):
    bf = ml_dtypes.bfloat16
    A_blk = np.zeros((IN, 2 * H), np.float32)
    for t in range(H):
        A_blk[t * D:(t + 1) * D, t] = attn_src[t]
        A_blk[t * D:(t + 1) * D, H + t] = attn_dst[t]
    P = W_proj.T.astype(np.float32) @ A_blk                      # [256, 8]
    P_own = np.zeros((IN, 128), np.float32)
    for t in range(H):
        P_own[:, 32 * t] = P[:, t]                               # src col at 32t
    wtp_full = np.concatenate([W_proj.T.astype(np.float32), P, P_own], axis=1)
    wtp_c = wtp_full.reshape(2, 128, IN + 8 + 128).copy()

    in_maps = []
    for core in range(8):
        b, q = core // 4, core % 4
        i0 = q * IBLK
        xb_T = np.ascontiguousarray(x[b].T)                      # [256, 4096]
        adjT_c = np.ascontiguousarray(adj[b, i0:i0 + IBLK, :].T.astype(bf))
        in_maps.append({
            "adjT": adjT_c.reshape(CH, 128, IBLK),
            "xT": xb_T.reshape(2, 128, N).copy(),
            "xTo": np.ascontiguousarray(xb_T[:, i0:i0 + IBLK]).reshape(2, 128, IBLK).copy(),
            "wtp": wtp_c,
        })
    return in_maps


def kernel(x, adj, W_proj, attn_src, attn_dst):
    global LAST_RESULT
    x = np.asarray(x, np.float32)
    adj = np.asarray(adj)
    W_proj = np.asarray(W_proj, np.float32)
    attn_src = np.asarray(attn_src, np.float32)
    attn_dst = np.asarray(attn_dst, np.float32)

    nc = _get_bass()
    in_maps = _prep_inputs(x, adj, W_proj, attn_src, attn_dst)
    br = run_bass_kernel_spmd(nc, in_maps, core_ids=list(range(8)))
    LAST_RESULT = br

    out = np.empty((B, N, H * D), np.float32)
    for core in range(8):
        b, q = core // 4, core % 4
        i0 = q * IBLK
        out[b, i0:i0 + IBLK, :] = br.results[core]["outT"].T
    return out

